# revision 26
# baseline (speedup 1.0000x reference)
"""Trainium2 Bass kernel for DynamicViewSampler.

Per sample b (of B=16): spotlight weights m[v,l] = exp(-20*dist2(center_v,
coord_l)) * (l < v_len[b]); out[b,v,:] = (m @ v_pad[b]) / (sum_l m + 1e-6).

Strategy (ragged_sequence): m is exactly 0 for l >= v_len[b], so only
ceil(v_len[b]/128) l-tiles of work exist per sample.  The host (this file)
reads v_len, packs the valid 128-row l-tiles into per-core groups (the
single SPMD program is identical across the 8 cores; all per-core variation
is carried by the packed input data), and pre-casts v_pad to bf16 (halves
DMA, and bf16 matmul streams 1 row/cycle on the PE vs 4 for fp32).  Group
sizes are a static per-slot vector shared by all cores — mostly S big slots
plus one small tail slot, chosen so ceil-padding is nearly zero.

On device, per l-tile (layout: l on partitions):
  - tiny K=3 fp32 matmul:  psum_c[l,v] = x_l*cx_v + y_l*cy_v - (cx_v^2+cy_v^2)/2
  - ACT:                   m[l,v] = Exp(40*psum_c + bias_l),
                           bias_l = -20*(x_l^2+y_l^2)   (or -1e5 -> m=0 for
                           invalid/padding rows: this realizes the ragged
                           mask and all padding)
  - 8 bf16 matmuls:        num[d,v] += v_tile[l, d_chunk].T @ m[l,v]
  - 1 bf16 matmul (ones):  den[v]   += 1.T @ m[l,v]
accumulated in PSUM over the tiles of a group (one group = one contiguous
chunk of one sample's tiles).  One fused DMA per group loads the group's
v-tiles; partials stage in SBUF and flush every two groups so stores overlap
compute.  Host sums the per-group partials and divides.
"""

import math

import numpy as np
import ml_dtypes

GAMMA = 20.0
P = 128
NCORES = 8
NEG_BIG = -1e5  # exp(40*psum + NEG_BIG) == 0.0 in fp32 for any |psum| ~ O(1)

# knobs (test.py may override)
REPLICAS = 1          # >1: repeat the whole compute for differential timing
LOOP_N = 1            # >1: wrap the body in a hardware For_i loop (timing)
FORCE_S = 5           # HW A/B: S=5 + small tail beat S=3 and S=7 configs
OUT_F32 = False       # numerator partials dtype (bf16 halves out-DMA)
ALT_QUEUES = False    # SP+ACT queue alternation measured slower on HW
VBUFS = 4             # v-data prefetch depth (groups in flight)
VAR_TAIL = True       # allow a smaller tail slot (cuts ceil-padding)

LAST = {}             # debug/timing info from the most recent kernel() call

_BUILD_CACHE = {}


# ----------------------------------------------------------------- planning

def _eff_grid(v_len, grid_thws):
    """Replicate reference W_eff/H_eff in float32-exact numpy."""
    Lv = v_len.astype(np.float32)
    H = grid_thws[:, 1].astype(np.float32)
    W = grid_thws[:, 2].astype(np.float32)
    W_eff = np.maximum(1, np.round(np.sqrt(Lv * (W / H))).astype(np.int32))
    H_eff = np.maximum(
        1, np.ceil(Lv / W_eff.astype(np.float32)).astype(np.int32)
    )
    return W_eff, H_eff


def _plan(v_len):
    """Choose static per-slot sizes and assign sample tile-chunks.

    All cores run the same slot-size vector sizes[0..G-1]; slots[c][g] is
    (sample, first_tile, n_real) or None (fully masked dummy).  A sample's
    tiles are split into chunks of at most sizes[g]; every slot's full
    sizes[g] tiles are processed (masked where not real), so the static
    program is identical across cores.
    """
    nt = np.maximum(1, (v_len.astype(np.int64) + P - 1) // P)
    total = int(nt.sum())

    best = None
    s_candidates = [FORCE_S] if FORCE_S else list(range(2, 9))
    for S in s_candidates:
        tails = range(0, S) if VAR_TAIL else [0]
        for s_tail in tails:  # 0 = uniform (no small tail slot)
            nbig = 0
            nsmall_cand = 0
            for n in nt:
                nbig += int(n // S)
                r = int(n % S)
                if r == 0:
                    continue
                if 0 < r <= s_tail:
                    nsmall_cand += 1
                else:
                    nbig += 1
            extra_small = max(0, nsmall_cand - NCORES) if s_tail else 0
            nbig += extra_small
            if s_tail:
                Gbig = (nbig + NCORES - 1) // NCORES
                G = Gbig + 1
                cap = Gbig * S + s_tail
            else:
                G = (nbig + NCORES - 1) // NCORES
                cap = G * S
            # per-core model (ns): 728/tile in-DMA, ~550/group overhead+out
            cost = cap * 728 + G * 550
            if best is None or cost < best[0]:
                best = (cost, S, s_tail, G)
    _, S, s_tail, G = best

    sizes = [S] * (G - 1) + [s_tail] if s_tail else [S] * G
    big_chunks, small_chunks = [], []
    for b in np.argsort(-nt):
        n = int(nt[b])
        k0 = 0
        while k0 < n:
            rem = n - k0
            if s_tail and 0 < rem <= s_tail and len(small_chunks) < NCORES:
                small_chunks.append((int(b), k0, rem))
                k0 = n
            else:
                take = min(S, rem)
                big_chunks.append((int(b), k0, take))
                k0 += take
    slots = [[None] * G for _ in range(NCORES)]
    nbig_slots = G - 1 if s_tail else G
    assert len(big_chunks) <= NCORES * nbig_slots, (len(big_chunks), G)
    for i, grp in enumerate(big_chunks):
        slots[i % NCORES][i // NCORES] = grp
    for c, grp in enumerate(small_chunks):
        slots[c][G - 1] = grp

    toff = np.concatenate([[0], np.cumsum(sizes)]).astype(int)  # tile offset
    plan = {
        "sizes": sizes, "slots": slots, "G": G, "TT": int(toff[-1]),
        "toff": toff, "maxS": max(sizes), "total": total,
    }
    return plan


# ------------------------------------------------------------- host packing

def _pack(v_pad, v_len, grid_thws, centers, plan):
    B, L, D = v_pad.shape
    V = centers.shape[1]
    sizes, slots, G, TT, toff = (plan["sizes"], plan["slots"], plan["G"],
                                 plan["TT"], plan["toff"])
    W_eff, H_eff = _eff_grid(v_len, grid_thws)

    v16 = v_pad.astype(ml_dtypes.bfloat16)  # one bulk cast
    vtot = P * TT * D

    in_maps = []
    for c in range(NCORES):
        vp = np.zeros(vtot, dtype=ml_dtypes.bfloat16)
        cw = np.zeros((4, TT * P), dtype=np.float32)
        cw[3, :] = np.float32(NEG_BIG / (2 * GAMMA))  # masked unless real
        cr = np.zeros((4, G * V), dtype=np.float32)
        cr[3, :] = 1.0  # bias row coefficient (also masks dummy groups)
        for g in range(G):
            slot = slots[c][g]
            if slot is None:
                continue
            b, k0, n_real = slot
            sz = sizes[g]
            cx = centers[b, :, 0].astype(np.float32)
            cy = centers[b, :, 1].astype(np.float32)
            cr[0, g * V:(g + 1) * V] = cx
            cr[1, g * V:(g + 1) * V] = cy
            cr[2, g * V:(g + 1) * V] = -(cx * cx + cy * cy) / np.float32(2.0)
            We = np.int32(W_eff[b])
            He_f = np.float32(H_eff[b])
            We_f = np.float32(We)
            block = vp[P * toff[g] * D:P * toff[g + 1] * D].reshape(P, sz * D)
            for j in range(n_real):
                t = toff[g] + j
                k = k0 + j
                l = np.arange(k * P, (k + 1) * P, dtype=np.int32)
                x = (l % We).astype(np.float32) / We_f
                y = (l // We).astype(np.float32) / He_f
                block[:, j * D:(j + 1) * D] = v16[b, k * P:(k + 1) * P, :]
                cw[0, t * P:(t + 1) * P] = x
                cw[1, t * P:(t + 1) * P] = y
                cw[2, t * P:(t + 1) * P] = 1.0
                valid = l < v_len[b]
                bias = -GAMMA * (x * x + y * y) / (2 * GAMMA)
                cw[3, t * P:(t + 1) * P] = np.where(
                    valid, bias.astype(np.float32),
                    np.float32(NEG_BIG / (2 * GAMMA)))
        in_maps.append({"vp": vp, "cw": cw, "cr": cr})
    return in_maps


# ------------------------------------------------------------ device kernel

def _build(plan, D, V, replicas):
    sizes, G, TT, toff = plan["sizes"], plan["G"], plan["TT"], plan["toff"]
    key = (tuple(sizes), D, V, replicas, OUT_F32, LOOP_N, ALT_QUEUES, VBUFS)
    if key in _BUILD_CACHE:
        return _BUILD_CACHE[key]

    import concourse.bass as bass  # noqa: F401
    import concourse.tile as tile
    from concourse import bacc, mybir

    f32 = mybir.dt.float32
    bf16 = mybir.dt.bfloat16
    out_dt = f32 if OUT_F32 else bf16
    NCH = D // P  # 8 d-chunks of 128
    NV = NCH * V  # 512 numerator columns per group

    nc = bacc.Bacc("TRN2", target_bir_lowering=False, debug=False,
                   num_devices=NCORES)
    vp = nc.dram_tensor("vp", [P * TT * D], bf16, kind="ExternalInput")
    cw = nc.dram_tensor("cw", [4, TT * P], f32, kind="ExternalInput")
    cr = nc.dram_tensor("cr", [4, G * V], f32, kind="ExternalInput")
    on = nc.dram_tensor("on", [P, G * NV], out_dt, kind="ExternalOutput")
    od = nc.dram_tensor("od", [1, G * V], f32, kind="ExternalOutput")

    Exp = mybir.ActivationFunctionType.Exp

    with tile.TileContext(nc) as tc:
        with (
            tc.tile_pool(name="singles", bufs=1) as singles,
            tc.tile_pool(name="vpool", bufs=VBUFS) as vpool,
            tc.tile_pool(name="mpool", bufs=3) as mpool,
            tc.tile_pool(name="stage", bufs=3) as stpool,
            tc.tile_pool(name="psc", bufs=2, space="PSUM") as psc,
            tc.tile_pool(name="psm", bufs=3, space="PSUM") as psm,
        ):
            cw_sb = singles.tile([4, TT * P], f32)
            nc.sync.dma_start(cw_sb, cw[:, :])
            cr_sb = singles.tile([4, G * V], f32)
            nc.sync.dma_start(cr_sb, cr[:, :])
            ones_sb = singles.tile([P, 1], bf16)
            nc.any.memset(ones_sb, 1.0)
            # warm the ACT exp table (1.3us load) off the critical path
            warm = singles.tile([1, 1], f32)
            nc.any.memset(warm, 0.0)
            nc.scalar.activation(warm, warm, Exp)

            import contextlib
            loop_ctx = (
                tc.For_i(0, LOOP_N, 1,
                         hint_engines=(mybir.EngineType.PE,
                                       mybir.EngineType.SP,
                                       mybir.EngineType.Activation,
                                       mybir.EngineType.DVE))
                if LOOP_N > 1 else contextlib.nullcontext()
            )
            with loop_ctx:
              for _r in range(replicas):
                stage_n = stage_d = None
                for g in range(G):
                    sz = sizes[g]
                    if stage_n is None:
                        stage_n = stpool.tile([P, 2 * NV], out_dt)
                        stage_d = stpool.tile([1, 2 * V], f32)
                        gbase = g
                    vg = vpool.tile([P, sz * D], bf16)
                    src = vp[P * toff[g] * D:P * toff[g + 1] * D].rearrange(
                        "(p f) -> p f", p=P)
                    # alternate load queues so HWDGE issue holds pipeline
                    eng = nc.sync if (g % 2 == 0 or not ALT_QUEUES) else nc.scalar
                    eng.dma_start(vg, src)
                    # one psum tile holds the whole group's coords dots
                    # (bias folded in as the 4th contraction row), exp'd in
                    # a single batched ACT instruction
                    ps_c = psc.tile([P, sz * V], f32)
                    for j in range(sz):
                        t = toff[g] + j
                        nc.tensor.matmul(
                            ps_c[:, j * V:(j + 1) * V],
                            lhsT=cw_sb[:, t * P:(t + 1) * P],
                            rhs=cr_sb[:, g * V:(g + 1) * V],
                            start=True, stop=True,
                        )
                    m_all = mpool.tile([P, sz * V], bf16)
                    nc.scalar.activation(
                        m_all, ps_c, Exp, scale=2.0 * GAMMA)
                    ps_main = psm.tile([P, NV + V], f32)
                    # chunk-major so each psum-bank accumulation group
                    # closes before the next opens (one pending group per
                    # 2KB zero-region); the denominator accumulates
                    # concurrently in the tile's second bank.
                    for ch in range(NCH):
                        for j in range(sz):
                            nc.tensor.matmul(
                                ps_main[:, ch * V:(ch + 1) * V],
                                lhsT=vg[:, j * D + ch * P:j * D + (ch + 1) * P],
                                rhs=m_all[:, j * V:(j + 1) * V],
                                start=(j == 0), stop=(j == sz - 1),
                            )
                    for j in range(sz):
                        nc.tensor.matmul(
                            ps_main[0:1, NV:NV + V],
                            lhsT=ones_sb,
                            rhs=m_all[:, j * V:(j + 1) * V],
                            start=(j == 0), stop=(j == sz - 1),
                        )
                    k = g - gbase
                    nc.vector.tensor_copy(
                        stage_n[:, k * NV:(k + 1) * NV], ps_main[:, 0:NV])
                    nc.vector.tensor_copy(
                        stage_d[0:1, k * V:(k + 1) * V],
                        ps_main[0:1, NV:NV + V])
                    # store DMAs ride the otherwise-idle gpsimd (SWDGE)
                    # queue: the in-order SP queue would head-of-line-block
                    # group loads behind a store that waits on copies.
                    if k == 1 or g >= G - 2:
                        nc.gpsimd.dma_start(
                            on[:, gbase * NV:(g + 1) * NV],
                            stage_n[:, 0:(k + 1) * NV])
                        nc.gpsimd.dma_start(
                            od[:, gbase * V:(g + 1) * V],
                            stage_d[0:1, 0:(k + 1) * V])
                        stage_n = stage_d = None

    nc.compile()
    _BUILD_CACHE[key] = nc
    return nc


# ------------------------------------------------------------------ driver

def _enable_jax_cache():
    """Persistent XLA/NEFF compile cache: a fresh process re-running the
    same geometry skips the ~30s neuronx compile."""
    try:
        import jax

        jax.config.update("jax_compilation_cache_dir", "/tmp/jax_nrt_cache")
        jax.config.update("jax_persistent_cache_min_compile_time_secs", 0.0)
    except Exception:
        pass


def kernel(v_pad, v_len, grid_thws, centers):
    import time as _time

    from concourse.bass_utils import run_bass_kernel_spmd

    _enable_jax_cache()

    v_pad = np.asarray(v_pad)
    v_len = np.asarray(v_len)
    grid_thws = np.asarray(grid_thws)
    centers = np.asarray(centers)

    B, L, D = v_pad.shape
    V = centers.shape[1]

    t0 = _time.monotonic()
    plan = _plan(v_len)
    in_maps = _pack(v_pad, v_len, grid_thws, centers, plan)
    t1 = _time.monotonic()
    nc = _build(plan, D, V, REPLICAS)
    t2 = _time.monotonic()
    res = run_bass_kernel_spmd(nc, in_maps, core_ids=list(range(NCORES)))
    t3 = _time.monotonic()

    G = plan["G"]
    slots = plan["slots"]
    NCH = D // P
    NV = NCH * V
    num = np.zeros((B, D, V), dtype=np.float32)
    den = np.zeros((B, V), dtype=np.float32)
    for c in range(NCORES):
        on = np.asarray(res.results[c]["on"], dtype=np.float32)
        od = np.asarray(res.results[c]["od"], dtype=np.float32)
        for g in range(G):
            slot = slots[c][g]
            if slot is None:
                continue
            b = slot[0]
            # on[p, g*NV + ch*V + v] == num[d=ch*P+p, v]
            num[b] += (on[:, g * NV:(g + 1) * NV]
                       .reshape(P, NCH, V).swapaxes(0, 1).reshape(D, V))
            den[b] += od[0, g * V:(g + 1) * V]
    out = (num / (den + np.float32(1e-6))[:, None, :]).swapaxes(1, 2)
    t4 = _time.monotonic()

    LAST.update(
        plan=plan, nc=nc, res=res,
        pack_s=t1 - t0, build_s=t2 - t1, run_s=t3 - t2, gather_s=t4 - t3,
    )
    return np.ascontiguousarray(out.astype(np.float32))


# revision 30
# speedup vs baseline: 1.2142x; 1.2142x over previous
"""Trainium2 Bass kernel for DynamicViewSampler.

Per sample b (of B=16): spotlight weights m[v,l] = exp(-20*dist2(center_v,
coord_l)) * (l < v_len[b]); out[b,v,:] = (m @ v_pad[b]) / (sum_l m + 1e-6).

Strategy (ragged_sequence): m is exactly 0 for l >= v_len[b], so only
ceil(v_len[b]/128) l-tiles of work exist per sample.  The host (this file)
reads v_len, packs the valid 128-row l-tiles into per-core groups (the
single SPMD program is identical across the 8 cores; all per-core variation
is carried by the packed input data), and pre-casts v_pad to bf16 (halves
DMA, and bf16 matmul streams 1 row/cycle on the PE vs 4 for fp32).  Group
sizes are a static per-slot vector shared by all cores — mostly S big slots
plus one small tail slot, chosen so ceil-padding is nearly zero.

On device, per l-tile (layout: l on partitions):
  - tiny K=4 fp32 matmul:  psum_c[l,v] = x_l*cx_v + y_l*cy_v
                           - (cx_v^2+cy_v^2)/2 - (x_l^2+y_l^2)/2
                           (rows: x, y, 1, bias; the bias row is -1e5/40 for
                           invalid/padding rows -> m = exp(-1e5) = 0, which
                           realizes the ragged mask and all padding)
  - one batched ACT/group: m[l,v] = Exp(40*psum_c)
  - 8 bf16 matmuls:        num[d,v] += v_tile[l, d_chunk].T @ m[l,v]
  - 1 bf16 matmul (ones):  den[v]   += 1.T @ m[l,v]
accumulated in PSUM over the tiles of a group (one group = one contiguous
chunk of one sample's tiles).  One fused DMA per group loads the group's
v-tiles; partials stage in SBUF and flush every two groups so stores overlap
compute.  Host sums the per-group partials and divides.
"""

import math

import numpy as np
import ml_dtypes

GAMMA = 20.0
P = 128
NCORES = 8
NEG_BIG = -1e5  # exp(40*psum + NEG_BIG) == 0.0 in fp32 for any |psum| ~ O(1)

# knobs (test.py may override)
REPLICAS = 1          # >1: repeat the whole compute for differential timing
LOOP_N = 1            # >1: wrap the body in a hardware For_i loop (timing)
FORCE_S = 5           # HW A/B: S=5 + small tail beat S=3 and S=7 configs
OUT_F32 = False       # numerator partials dtype (bf16 halves out-DMA)
ALT_QUEUES = False    # SP+ACT queue alternation measured slower on HW
VBUFS = 4             # v-data prefetch depth (groups in flight)
VAR_TAIL = True       # allow a smaller tail slot (cuts ceil-padding)
ORIENT = "vd"         # "vd": m stationary (1 LDW/tile, N=512 streams, out[v,d])
                      # "dv": v stationary (9 LDW/tile, N=64 streams, out[d,v])

LAST = {}             # debug/timing info from the most recent kernel() call

_BUILD_CACHE = {}


# ----------------------------------------------------------------- planning

def _eff_grid(v_len, grid_thws):
    """Replicate reference W_eff/H_eff in float32-exact numpy."""
    Lv = v_len.astype(np.float32)
    H = grid_thws[:, 1].astype(np.float32)
    W = grid_thws[:, 2].astype(np.float32)
    W_eff = np.maximum(1, np.round(np.sqrt(Lv * (W / H))).astype(np.int32))
    H_eff = np.maximum(
        1, np.ceil(Lv / W_eff.astype(np.float32)).astype(np.int32)
    )
    return W_eff, H_eff


def _plan(v_len):
    """Choose static per-slot sizes and assign sample tile-chunks.

    All cores run the same slot-size vector sizes[0..G-1]; slots[c][g] is
    (sample, first_tile, n_real) or None (fully masked dummy).  A sample's
    tiles are split into chunks of at most sizes[g]; every slot's full
    sizes[g] tiles are processed (masked where not real), so the static
    program is identical across cores.
    """
    nt = np.maximum(1, (v_len.astype(np.int64) + P - 1) // P)
    total = int(nt.sum())

    best = None
    s_candidates = [FORCE_S] if FORCE_S else list(range(2, 9))
    for S in s_candidates:
        tails = range(0, S) if VAR_TAIL else [0]
        for s_tail in tails:  # 0 = uniform (no small tail slot)
            nbig = 0
            nsmall_cand = 0
            for n in nt:
                nbig += int(n // S)
                r = int(n % S)
                if r == 0:
                    continue
                if 0 < r <= s_tail:
                    nsmall_cand += 1
                else:
                    nbig += 1
            extra_small = max(0, nsmall_cand - NCORES) if s_tail else 0
            nbig += extra_small
            if s_tail:
                Gbig = (nbig + NCORES - 1) // NCORES
                G = Gbig + 1
                cap = Gbig * S + s_tail
            else:
                G = (nbig + NCORES - 1) // NCORES
                cap = G * S
            # per-core model (ns): 728/tile in-DMA, ~550/group overhead+out
            cost = cap * 728 + G * 550
            if best is None or cost < best[0]:
                best = (cost, S, s_tail, G)
    _, S, s_tail, G = best

    sizes = [S] * (G - 1) + [s_tail] if s_tail else [S] * G
    big_chunks, small_chunks = [], []
    for b in np.argsort(-nt):
        n = int(nt[b])
        k0 = 0
        while k0 < n:
            rem = n - k0
            if s_tail and 0 < rem <= s_tail and len(small_chunks) < NCORES:
                small_chunks.append((int(b), k0, rem))
                k0 = n
            else:
                take = min(S, rem)
                big_chunks.append((int(b), k0, take))
                k0 += take
    slots = [[None] * G for _ in range(NCORES)]
    nbig_slots = G - 1 if s_tail else G
    assert len(big_chunks) <= NCORES * nbig_slots, (len(big_chunks), G)
    for i, grp in enumerate(big_chunks):
        slots[i % NCORES][i // NCORES] = grp
    for c, grp in enumerate(small_chunks):
        slots[c][G - 1] = grp

    toff = np.concatenate([[0], np.cumsum(sizes)]).astype(int)  # tile offset
    plan = {
        "sizes": sizes, "slots": slots, "G": G, "TT": int(toff[-1]),
        "toff": toff, "maxS": max(sizes), "total": total,
    }
    return plan


# ------------------------------------------------------------- host packing

def _pack(v_pad, v_len, grid_thws, centers, plan):
    B, L, D = v_pad.shape
    V = centers.shape[1]
    sizes, slots, G, TT, toff = (plan["sizes"], plan["slots"], plan["G"],
                                 plan["TT"], plan["toff"])
    W_eff, H_eff = _eff_grid(v_len, grid_thws)

    v16 = v_pad.astype(ml_dtypes.bfloat16)  # one bulk cast
    vtot = P * TT * D

    in_maps = []
    for c in range(NCORES):
        vp = np.zeros(vtot, dtype=ml_dtypes.bfloat16)
        cw = np.zeros((4, TT * P), dtype=np.float32)
        cw[3, :] = np.float32(NEG_BIG / (2 * GAMMA))  # masked unless real
        cr = np.zeros((4, G * V), dtype=np.float32)
        cr[3, :] = 1.0  # bias row coefficient (also masks dummy groups)
        for g in range(G):
            slot = slots[c][g]
            if slot is None:
                continue
            b, k0, n_real = slot
            sz = sizes[g]
            cx = centers[b, :, 0].astype(np.float32)
            cy = centers[b, :, 1].astype(np.float32)
            cr[0, g * V:(g + 1) * V] = cx
            cr[1, g * V:(g + 1) * V] = cy
            cr[2, g * V:(g + 1) * V] = -(cx * cx + cy * cy) / np.float32(2.0)
            We = np.int32(W_eff[b])
            He_f = np.float32(H_eff[b])
            We_f = np.float32(We)
            block = vp[P * toff[g] * D:P * toff[g + 1] * D].reshape(P, sz * D)
            for j in range(n_real):
                t = toff[g] + j
                k = k0 + j
                l = np.arange(k * P, (k + 1) * P, dtype=np.int32)
                x = (l % We).astype(np.float32) / We_f
                y = (l // We).astype(np.float32) / He_f
                block[:, j * D:(j + 1) * D] = v16[b, k * P:(k + 1) * P, :]
                cw[0, t * P:(t + 1) * P] = x
                cw[1, t * P:(t + 1) * P] = y
                cw[2, t * P:(t + 1) * P] = 1.0
                valid = l < v_len[b]
                bias = -GAMMA * (x * x + y * y) / (2 * GAMMA)
                cw[3, t * P:(t + 1) * P] = np.where(
                    valid, bias.astype(np.float32),
                    np.float32(NEG_BIG / (2 * GAMMA)))
        in_maps.append({"vp": vp, "cw": cw, "cr": cr})
    return in_maps


# ------------------------------------------------------------ device kernel

def _build(plan, D, V, replicas):
    sizes, G, TT, toff = plan["sizes"], plan["G"], plan["TT"], plan["toff"]
    key = (tuple(sizes), D, V, replicas, OUT_F32, LOOP_N, ALT_QUEUES, VBUFS,
           ORIENT)
    if key in _BUILD_CACHE:
        return _BUILD_CACHE[key]

    import concourse.bass as bass  # noqa: F401
    import concourse.tile as tile
    from concourse import bacc, mybir

    f32 = mybir.dt.float32
    bf16 = mybir.dt.bfloat16
    out_dt = f32 if OUT_F32 else bf16
    NCH = D // P  # 8 d-chunks of 128
    NV = NCH * V  # 512 numerator columns per group

    nc = bacc.Bacc("TRN2", target_bir_lowering=False, debug=False,
                   num_devices=NCORES)
    vp = nc.dram_tensor("vp", [P * TT * D], bf16, kind="ExternalInput")
    cw = nc.dram_tensor("cw", [4, TT * P], f32, kind="ExternalInput")
    cr = nc.dram_tensor("cr", [4, G * V], f32, kind="ExternalInput")
    if ORIENT == "vd":
        on = nc.dram_tensor("on", [V, G * D], out_dt, kind="ExternalOutput")
        od = nc.dram_tensor("od", [V, G], f32, kind="ExternalOutput")
    else:
        on = nc.dram_tensor("on", [P, G * NV], out_dt, kind="ExternalOutput")
        od = nc.dram_tensor("od", [1, G * V], f32, kind="ExternalOutput")

    Exp = mybir.ActivationFunctionType.Exp

    with tile.TileContext(nc) as tc:
        with (
            tc.tile_pool(name="singles", bufs=1) as singles,
            tc.tile_pool(name="vpool", bufs=VBUFS) as vpool,
            tc.tile_pool(name="mpool", bufs=3) as mpool,
            tc.tile_pool(name="stage", bufs=3) as stpool,
            tc.tile_pool(name="psc", bufs=2, space="PSUM") as psc,
            # vd psum tile is [64, D+1] = 3 banks; 2 bufs + psc 2 = 8 banks
            tc.tile_pool(name="psm", bufs=2 if ORIENT == "vd" else 3,
                         space="PSUM") as psm,
        ):
            cw_sb = singles.tile([4, TT * P], f32)
            nc.sync.dma_start(cw_sb, cw[:, :])
            cr_sb = singles.tile([4, G * V], f32)
            nc.sync.dma_start(cr_sb, cr[:, :])
            ones_sb = singles.tile([P, 1], bf16)
            nc.any.memset(ones_sb, 1.0)
            # warm the ACT exp table (1.3us load) off the critical path
            warm = singles.tile([1, 1], f32)
            nc.any.memset(warm, 0.0)
            nc.scalar.activation(warm, warm, Exp)

            import contextlib
            loop_ctx = (
                tc.For_i(0, LOOP_N, 1,
                         hint_engines=(mybir.EngineType.PE,
                                       mybir.EngineType.SP,
                                       mybir.EngineType.Activation,
                                       mybir.EngineType.DVE))
                if LOOP_N > 1 else contextlib.nullcontext()
            )
            with loop_ctx:
              for _r in range(replicas):
                stage_n = stage_d = None
                for g in range(G):
                    sz = sizes[g]
                    if stage_n is None:
                        if ORIENT == "vd":
                            stage_n = stpool.tile([V, 2 * D], out_dt)
                            stage_d = stpool.tile([V, 2], f32)
                        else:
                            stage_n = stpool.tile([P, 2 * NV], out_dt)
                            stage_d = stpool.tile([1, 2 * V], f32)
                        gbase = g
                    vg = vpool.tile([P, sz * D], bf16)
                    src = vp[P * toff[g] * D:P * toff[g + 1] * D].rearrange(
                        "(p f) -> p f", p=P)
                    # alternate load queues so HWDGE issue holds pipeline
                    eng = nc.sync if (g % 2 == 0 or not ALT_QUEUES) else nc.scalar
                    eng.dma_start(vg, src)
                    # one psum tile holds the whole group's coords dots
                    # (bias folded in as the 4th contraction row), exp'd in
                    # a single batched ACT instruction
                    ps_c = psc.tile([P, sz * V], f32)
                    for j in range(sz):
                        t = toff[g] + j
                        nc.tensor.matmul(
                            ps_c[:, j * V:(j + 1) * V],
                            lhsT=cw_sb[:, t * P:(t + 1) * P],
                            rhs=cr_sb[:, g * V:(g + 1) * V],
                            start=True, stop=True,
                        )
                    m_all = mpool.tile([P, sz * V], bf16)
                    nc.scalar.activation(
                        m_all, ps_c, Exp, scale=2.0 * GAMMA)
                    k = g - gbase
                    if ORIENT == "vd":
                        # m is the stationary operand: one small LDW per
                        # tile, v streams through as two N=512 matmuls, so
                        # the PE spends its cycles streaming rather than
                        # reloading weights.  num in banks 0-1, den column
                        # in bank 2 — three concurrently-pending
                        # accumulation groups in distinct zero-regions.
                        ps_main = psm.tile([V, D + 1], f32)
                        for j in range(sz):
                            mw = m_all[:, j * V:(j + 1) * V]
                            for h in range(D // 512):
                                nc.tensor.matmul(
                                    ps_main[:, h * 512:(h + 1) * 512],
                                    lhsT=mw,
                                    rhs=vg[:, j * D + h * 512:
                                           j * D + (h + 1) * 512],
                                    start=(j == 0), stop=(j == sz - 1),
                                )
                            nc.tensor.matmul(
                                ps_main[:, D:D + 1],
                                lhsT=mw, rhs=ones_sb,
                                start=(j == 0), stop=(j == sz - 1),
                            )
                        nc.vector.tensor_copy(
                            stage_n[:, k * D:(k + 1) * D], ps_main[:, 0:D])
                        nc.vector.tensor_copy(
                            stage_d[:, k:k + 1], ps_main[:, D:D + 1])
                    else:
                        ps_main = psm.tile([P, NV + V], f32)
                        # chunk-major so each psum-bank accumulation group
                        # closes before the next opens (one pending group
                        # per 2KB zero-region); the denominator accumulates
                        # concurrently in the tile's second bank.
                        for ch in range(NCH):
                            for j in range(sz):
                                nc.tensor.matmul(
                                    ps_main[:, ch * V:(ch + 1) * V],
                                    lhsT=vg[:, j * D + ch * P:
                                            j * D + (ch + 1) * P],
                                    rhs=m_all[:, j * V:(j + 1) * V],
                                    start=(j == 0), stop=(j == sz - 1),
                                )
                        for j in range(sz):
                            nc.tensor.matmul(
                                ps_main[0:1, NV:NV + V],
                                lhsT=ones_sb,
                                rhs=m_all[:, j * V:(j + 1) * V],
                                start=(j == 0), stop=(j == sz - 1),
                            )
                        nc.vector.tensor_copy(
                            stage_n[:, k * NV:(k + 1) * NV], ps_main[:, 0:NV])
                        nc.vector.tensor_copy(
                            stage_d[0:1, k * V:(k + 1) * V],
                            ps_main[0:1, NV:NV + V])
                    # store DMAs ride the otherwise-idle gpsimd (SWDGE)
                    # queue: the in-order SP queue would head-of-line-block
                    # group loads behind a store that waits on copies.
                    if k == 1 or g >= G - 2:
                        if ORIENT == "vd":
                            nc.gpsimd.dma_start(
                                on[:, gbase * D:(g + 1) * D],
                                stage_n[:, 0:(k + 1) * D])
                            nc.gpsimd.dma_start(
                                od[:, gbase:g + 1],
                                stage_d[:, 0:k + 1])
                        else:
                            nc.gpsimd.dma_start(
                                on[:, gbase * NV:(g + 1) * NV],
                                stage_n[:, 0:(k + 1) * NV])
                            nc.gpsimd.dma_start(
                                od[:, gbase * V:(g + 1) * V],
                                stage_d[0:1, 0:(k + 1) * V])
                        stage_n = stage_d = None

    nc.compile()
    _BUILD_CACHE[key] = nc
    return nc


# ------------------------------------------------------------------ driver

def _enable_jax_cache():
    """Persistent XLA/NEFF compile cache: a fresh process re-running the
    same geometry skips the ~30s neuronx compile."""
    try:
        import jax

        jax.config.update("jax_compilation_cache_dir", "/tmp/jax_nrt_cache")
        jax.config.update("jax_persistent_cache_min_compile_time_secs", 0.0)
    except Exception:
        pass


def kernel(v_pad, v_len, grid_thws, centers):
    import time as _time

    from concourse.bass_utils import run_bass_kernel_spmd

    _enable_jax_cache()

    v_pad = np.asarray(v_pad)
    v_len = np.asarray(v_len)
    grid_thws = np.asarray(grid_thws)
    centers = np.asarray(centers)

    B, L, D = v_pad.shape
    V = centers.shape[1]

    t0 = _time.monotonic()
    plan = _plan(v_len)
    in_maps = _pack(v_pad, v_len, grid_thws, centers, plan)
    t1 = _time.monotonic()
    nc = _build(plan, D, V, REPLICAS)
    t2 = _time.monotonic()
    res = run_bass_kernel_spmd(nc, in_maps, core_ids=list(range(NCORES)))
    t3 = _time.monotonic()

    G = plan["G"]
    slots = plan["slots"]
    NCH = D // P
    NV = NCH * V
    den = np.zeros((B, V), dtype=np.float32)
    if ORIENT == "vd":
        num = np.zeros((B, V, D), dtype=np.float32)
        for c in range(NCORES):
            on = np.asarray(res.results[c]["on"], dtype=np.float32)
            od = np.asarray(res.results[c]["od"], dtype=np.float32)
            for g in range(G):
                slot = slots[c][g]
                if slot is None:
                    continue
                b = slot[0]
                num[b] += on[:, g * D:(g + 1) * D]
                den[b] += od[:, g]
        out = num / (den + np.float32(1e-6))[:, :, None]
    else:
        num = np.zeros((B, D, V), dtype=np.float32)
        for c in range(NCORES):
            on = np.asarray(res.results[c]["on"], dtype=np.float32)
            od = np.asarray(res.results[c]["od"], dtype=np.float32)
            for g in range(G):
                slot = slots[c][g]
                if slot is None:
                    continue
                b = slot[0]
                # on[p, g*NV + ch*V + v] == num[d=ch*P+p, v]
                num[b] += (on[:, g * NV:(g + 1) * NV]
                           .reshape(P, NCH, V).swapaxes(0, 1).reshape(D, V))
                den[b] += od[0, g * V:(g + 1) * V]
        out = (num / (den + np.float32(1e-6))[:, None, :]).swapaxes(1, 2)
    t4 = _time.monotonic()

    LAST.update(
        plan=plan, nc=nc, res=res,
        pack_s=t1 - t0, build_s=t2 - t1, run_s=t3 - t2, gather_s=t4 - t3,
    )
    return np.ascontiguousarray(out.astype(np.float32))
